# revision 17
# baseline (speedup 1.0000x reference)
"""Trainium2 Bass kernel for nn_FFMLP (4-layer MLP, hidden=128, relu).

Strategy (pure data parallel, batch sharded 8 ways):
- Feature-major on-chip layout: activations live as [feat, batch]; each layer
  is a K<=128 matmul with the tiny replicated weight stationary and the
  activation stream moving. fp16 operands, fp32 PSUM.
- Layer-batched slab schedule: per slab of 32 chunks, run all of L0, then all
  of L1, ... so consecutive PE matmuls share weights (one LdWeights per layer
  per slab instead of one per matmul -- the interleaved baseline paid ~50us
  of LDWEIGHTS on the PE queue).
- L0 (K=32) packs 4 chunks into concurrent row-tiled matmuls at
  tile_position (32i, 0); L4 (M=16) packs 4 chunks per PSUM bank via column
  tiling (0, 32j). Tiled matmuls execute concurrently on the PE.
- PSUM: 8 banks as two double-buffered 2-bank groups, one owned by the
  Scalar (ACT) engine, one by Vector (DVE). PSUM->SBUF relu evacuation is
  the structural bottleneck (only these two engines can read PSUM);
  a 6:5 ACT:DVE group pattern balances their 0.83 vs 1.10 ns/col rates and
  keeps both ~100% busy without bank starvation.
- Output is packed fp16 in a (group, bank, strip) lexicographic layout so the
  host just reshapes; final cast to fp32 on host.
"""
import sys

if "/opt/trn_rl_repo" not in sys.path:
    sys.path.insert(0, "/opt/trn_rl_repo")

import numpy as np

import concourse.bass as bass
import concourse.mybir as mybir
import concourse.tile as tile

INPUT_DIM = 32
OUTPUT_DIM = 16
HIDDEN = 128
PADDED_OUT = 16
NUM_LAYERS = 4
B = 524288
N_CORES = 8
B_CORE = B // N_CORES  # 65536
CHUNK = 512
N_CHUNKS = B_CORE // CHUNK  # 128
SLAB = 64  # chunks per slab (layer-batched unit)
N_SLABS = N_CHUNKS // SLAB  # 4
QUADS_PER_SLAB = SLAB // 4  # 8 (4-chunk quads for L0 row tiling)
GROUPS_PER_SLAB = SLAB // 2  # 16 2-chunk PSUM groups per layer phase
L4_GROUPS_PER_SLAB = SLAB // 8  # 4 (8 chunks of output per 2-bank group)
N_L4_GROUPS = N_CHUNKS // 8  # 16

fp16 = mybir.dt.float16
fp32 = mybir.dt.float32
RELU = mybir.ActivationFunctionType.Relu

# evac engine pattern: 6 ACT : 5 DVE matches the measured ~1020 : ~1205 ns
# per-op busy times (both engines' spans balance at ~157us)
EVAC_PAT = "ADADADADADA"


def _split_waits(nc, max_waits=1):
    """walrus in this image rejects >1 semaphore wait per instruction on some
    formats; split excess waits onto preceding NOPs on the same engine queue
    (queues are in-order, so semantics are preserved)."""
    n_new = 0
    for bb in nc.main_func.blocks:
        out_list = []
        changed = False
        for ins in bb.instructions:
            si = ins.sync_info
            if si is not None and si.on_wait and len(si.on_wait) > max_waits:
                waits = list(si.on_wait)
                extra, keep = waits[:-max_waits], waits[-max_waits:]
                while extra:
                    chunk, extra = extra[:max_waits], extra[max_waits:]
                    n_new += 1
                    nop = mybir.InstNoOp(name=f"I-waitsplit-{n_new}", ins=[], outs=[])
                    nop.engine = ins.engine
                    nop.sync_info = mybir.SyncInfo(on_wait=chunk, on_update=[])
                    out_list.append(nop)
                ins.sync_info = mybir.SyncInfo(on_wait=keep, on_update=si.on_update)
                changed = True
            out_list.append(ins)
        if changed:
            bb.instructions = out_list
    return n_new


def _ldw_rect(ins):
    """PE-array rectangle (r0, r1, c0, c1) occupied by an InstLdweights."""
    tp = ins.tile_position
    ts = getattr(ins, "tile_size", None)
    r0, c0 = (tp if tp else (0, 0))
    if ts:
        rows, cols = ts
    else:
        rows, cols = 128, 128
    return (r0, r0 + rows, c0, c0 + cols)


def _dedup_ldweights(nc):
    """Tile emits an explicit InstLdweights before every matmul. Weights at a
    given tile rectangle stay resident until an overlapping load clobbers
    them, so replace reloads of already-resident weights with NOPs (keeping
    sync_info). Tracks residency per array rectangle, which handles the
    alternating tile positions of the row/col-tiled L0/L4 phases."""
    n = 0
    for bb in nc.main_func.blocks:
        il = list(bb.instructions)
        live = {}  # rect -> content key
        changed = False
        for idx, ins in enumerate(il):
            if ins.engine != mybir.EngineType.PE:
                continue
            if isinstance(ins, mybir.InstLdweights):
                rect = _ldw_rect(ins)
                key = (
                    repr(ins.ins[0]),
                    str(ins.tile_position),
                    str(getattr(ins, "tile_size", None)),
                    str(ins.perf_mode),
                    bool(ins.is_transpose),
                )
                if live.get(rect) == key:
                    nop = mybir.InstNoOp(name=ins.name, ins=[], outs=[])
                    nop.engine = ins.engine
                    nop.sync_info = ins.sync_info
                    il[idx] = nop
                    changed = True
                    n += 1
                else:
                    r0, r1, c0, c1 = rect
                    for other in list(live):
                        o0, o1, p0, p1 = other
                        if r0 < o1 and o0 < r1 and c0 < p1 and p0 < c1:
                            del live[other]
                    live[rect] = key
        if changed:
            bb.instructions = il
    return n


def build(n_slabs=N_SLABS):
    nc = bass.Bass()
    n_chunks = n_slabs * SLAB
    # xt4: quad-strip layout -- xt4[32*i + f, q*CHUNK + c] = x.T[f, (4q+i)*CHUNK + c]
    # so each quad of 4 chunks feeds 4 concurrent row-tiled K=32 L0 matmuls.
    xt = nc.declare_dram_parameter(
        "xt", [4 * INPUT_DIM, n_chunks * CHUNK // 4], fp16, isOutput=False
    )
    wd = nc.declare_dram_parameter(
        "wd", [HIDDEN, 4 * HIDDEN + PADDED_OUT], fp16, isOutput=False
    )
    # yt[o, g, j, b, c] = y.T[o, (8g + 4b + j)*CHUNK + c] -- the (j, b) order
    # lets one rearranged DMA per L4 group write all 4 partition strips;
    # host transposes (0,1,3,2,4) and reshapes to [16, B_CORE].
    n_l4_groups = n_chunks // 8
    yt = nc.declare_dram_parameter(
        "yt", [PADDED_OUT, n_l4_groups, 4, 2, CHUNK], fp16, isOutput=True
    )

    with tile.TileContext(nc) as tc:
        with (
            tc.tile_pool(name="wp", bufs=1) as wp,
            tc.tile_pool(name="io", bufs=1) as io,
            tc.tile_pool(name="hp", bufs=1) as hp,
            tc.tile_pool(name="ps", bufs=1, space="PSUM") as ps,
        ):
            wall = wp.tile(
                [HIDDEN, 4 * HIDDEN + PADDED_OUT], fp16, tag="wall", name="wall"
            )
            nc.sync.dma_start(out=wall, in_=wd[:, :])
            w0s = wall[:, 0:HIDDEN]
            w1s = wall[:, HIDDEN : 2 * HIDDEN]
            w2s = wall[:, 2 * HIDDEN : 3 * HIDDEN]
            w3s = wall[:, 3 * HIDDEN : 4 * HIDDEN]
            w4s = wall[:, 4 * HIDDEN : 4 * HIDDEN + PADDED_OUT]

            # hidden activation ping-pong buffers, one slab each
            hA = hp.tile([HIDDEN, SLAB * CHUNK], fp16, tag="hA", name="hA")
            hB = hp.tile([HIDDEN, SLAB * CHUNK], fp16, tag="hB", name="hB")

            SLAB_COLS = SLAB * CHUNK // 4  # xt cols per slab (quad-packed)

            xs_tiles = {}

            def fetch_slab(s):
                if s >= n_slabs:
                    return
                xs = io.tile(
                    [4 * INPUT_DIM, SLAB_COLS], fp16, tag="xin", bufs=2, name="xs"
                )
                # quarter-DMAs so L0 of the first quads can start before the
                # whole slab lands (subtile deps gate on each DMA separately)
                q4 = SLAB_COLS // 4
                for k in range(4):
                    nc.sync.dma_start(
                        out=xs[:, k * q4 : (k + 1) * q4],
                        in_=xt[
                            :,
                            s * SLAB_COLS + k * q4 : s * SLAB_COLS + (k + 1) * q4,
                        ],
                    )
                xs_tiles[s] = xs

            fetch_slab(0)
            # short HAM warm-up while the first input quarter lands
            pwarm = ps.tile([HIDDEN, 2 * CHUNK], fp32, tag="pg", bufs=4, name="pwarm")
            for _ in range(8):
                nc.tensor.matmul(
                    pwarm[:, 0:HIDDEN], w1s[:, :], w2s[:, 0:HIDDEN],
                    start=True, stop=True,
                )
            fetch_slab(1)

            # greedy engine assignment by measured per-op busy cost (ns):
            # assign each group to whichever engine would finish it sooner
            evac_state = {"A": 0.0, "D": 0.0}
            COST = {("A", True): 1013, ("A", False): 975,
                    ("D", True): 1169, ("D", False): 1215}

            def evac(dst, src, relu):
                ca = COST[("A", relu)]
                cd = COST[("D", relu)]
                if evac_state["A"] + ca <= evac_state["D"] + cd:
                    evac_state["A"] += ca
                    if relu:
                        nc.scalar.activation(dst, src, RELU)
                    else:
                        nc.scalar.copy(out=dst, in_=src)
                else:
                    evac_state["D"] += cd
                    if relu:
                        nc.vector.tensor_scalar_max(dst, src, 0.0)
                    else:
                        nc.vector.tensor_copy(dst, src)

            def psum_group():
                # one 4-deep ring of 2-bank groups: the PE can run up to 4
                # groups ahead no matter which evac engine is lagging
                return ps.tile([HIDDEN, 2 * CHUNK], fp32, tag="pg", bufs=4, name="pg")

            def emit_l4(s, gg):
                # L4: M=16, 4-way col tiling packs 4 chunks/bank; a 2-bank
                # group holds 8 chunks of output.
                g = psum_group()
                for b in range(2):
                    for j in range(4):
                        c = 8 * gg + 4 * b + j
                        nc.tensor.matmul(
                            g[32 * j : 32 * j + PADDED_OUT, b * CHUNK : (b + 1) * CHUNK],
                            w4s[:, :],
                            hB[:, c * CHUNK : (c + 1) * CHUNK],
                            start=True, stop=True,
                            tile_position=(0, 32 * j),
                        )
                osb = io.tile([HIDDEN, 2 * CHUNK], fp16, tag="osb", bufs=8, name="osb")
                evac(osb[:, :], g[:, :], False)
                g_abs = s * L4_GROUPS_PER_SLAB + gg
                for j in range(4):
                    eng = nc.sync if j < 2 else nc.gpsimd
                    eng.dma_start(
                        out=yt[:, g_abs : g_abs + 1, j : j + 1, :, :],
                        in_=osb[32 * j : 32 * j + PADDED_OUT, :],
                    )

            for s in range(n_slabs):
                xs = xs_tiles.pop(s)
                # ---- L0: K=32, 4-way row tiling; one quad = 4 concurrent MMs
                # spanning two 2-bank groups. The previous slab's L4 groups
                # (which depend only on the old hB) are interleaved here so
                # the evac engines never starve at the slab boundary.
                for q in range(QUADS_PER_SLAB):
                    g0 = psum_group()
                    for i in range(2):
                        nc.tensor.matmul(
                            g0[:, i * CHUNK : (i + 1) * CHUNK],
                            w0s[32 * i : 32 * i + INPUT_DIM, :],
                            xs[32 * i : 32 * i + INPUT_DIM, q * CHUNK : (q + 1) * CHUNK],
                            start=True, stop=True,
                            tile_position=(32 * i, 0),
                        )
                    evac(hA[:, (4 * q) * CHUNK : (4 * q + 2) * CHUNK], g0[:, :], True)
                    g1 = psum_group()
                    for i in range(2, 4):
                        nc.tensor.matmul(
                            g1[:, (i - 2) * CHUNK : (i - 1) * CHUNK],
                            w0s[32 * i : 32 * i + INPUT_DIM, :],
                            xs[32 * i : 32 * i + INPUT_DIM, q * CHUNK : (q + 1) * CHUNK],
                            start=True, stop=True,
                            tile_position=(32 * i, 0),
                        )
                    evac(hA[:, (4 * q + 2) * CHUNK : (4 * q + 4) * CHUNK], g1[:, :], True)
                # prefetch input two slabs ahead (slabs 0/1 fetched up front)
                if s + 2 < n_slabs:
                    fetch_slab(s + 2)

                # ---- L1..L3: full-array matmuls, 2 chunks per PSUM group.
                # L4 group gg is interleaved into L3 two groups after its
                # source h4 groups (4gg..4gg+3) are emitted, keeping the evac
                # stream dense through the slab boundary and the kernel tail.
                for li, (ws, hin, hout) in enumerate(
                    ((w1s, hA, hB), (w2s, hB, hA), (w3s, hA, hB))
                ):
                    for t in range(GROUPS_PER_SLAB):
                        g = psum_group()
                        for k in range(2):
                            c = 2 * t + k
                            nc.tensor.matmul(
                                g[:, k * CHUNK : (k + 1) * CHUNK],
                                ws[:, :],
                                hin[:, c * CHUNK : (c + 1) * CHUNK],
                                start=True, stop=True,
                            )
                        evac(
                            hout[:, (2 * t) * CHUNK : (2 * t + 2) * CHUNK], g[:, :], True
                        )
                        if li == 2 and t >= 6 and (t - 6) % 4 == 0:
                            emit_l4(s, (t - 6) // 4)
                    if li == 2:
                        for gg in range((GROUPS_PER_SLAB - 6 + 3) // 4, L4_GROUPS_PER_SLAB):
                            emit_l4(s, gg)
    _dedup_ldweights(nc)
    _split_waits(nc)
    return nc


def _split_weights(weights):
    ws = []
    off = 0
    ws.append(weights[off : off + HIDDEN * INPUT_DIM].reshape(HIDDEN, INPUT_DIM))
    off += HIDDEN * INPUT_DIM
    for _ in range(NUM_LAYERS - 1):
        ws.append(weights[off : off + HIDDEN * HIDDEN].reshape(HIDDEN, HIDDEN))
        off += HIDDEN * HIDDEN
    ws.append(weights[off : off + PADDED_OUT * HIDDEN].reshape(PADDED_OUT, HIDDEN))
    return ws


_NC_CACHE = {}


def make_in_maps(inputs: np.ndarray, weights: np.ndarray):
    ws = _split_weights(np.asarray(weights, dtype=np.float32))
    # stationary operands are lhsT = [K_in, M_out] = W.T; W0.T is stacked
    # four times for the four row-tiled strips.
    w0t = np.ascontiguousarray(ws[0].T).astype(np.float16)
    wd = np.concatenate(
        [
            np.concatenate([w0t, w0t, w0t, w0t], axis=0),  # [128, 128]
            np.ascontiguousarray(ws[1].T).astype(np.float16),
            np.ascontiguousarray(ws[2].T).astype(np.float16),
            np.ascontiguousarray(ws[3].T).astype(np.float16),
            np.ascontiguousarray(ws[4].T).astype(np.float16),  # [128, 16]
        ],
        axis=1,
    )
    wmaps = {"wd": np.ascontiguousarray(wd)}
    in_maps = []
    for i in range(N_CORES):
        xc = inputs[i * B_CORE : (i + 1) * B_CORE]
        xtc = np.ascontiguousarray(xc.T).astype(np.float16)  # [32, B_CORE]
        # quad-strip layout: [128, B_CORE//4]
        xt4 = np.ascontiguousarray(
            xtc.reshape(INPUT_DIM, B_CORE // (4 * CHUNK), 4, CHUNK)
            .transpose(2, 0, 1, 3)
            .reshape(4 * INPUT_DIM, B_CORE // 4)
        )
        in_maps.append({"xt": xt4, **wmaps})
    return in_maps


def kernel(inputs: np.ndarray, weights: np.ndarray) -> np.ndarray:
    from concourse.bass_utils import run_bass_kernel_spmd

    assert inputs.shape == (B, INPUT_DIM), inputs.shape
    in_maps = make_in_maps(inputs, weights)
    if "nc" not in _NC_CACHE:
        _NC_CACHE["nc"] = build()
    nc = _NC_CACHE["nc"]
    res = run_bass_kernel_spmd(nc, in_maps, list(range(N_CORES)))
    outs = [
        np.ascontiguousarray(
            r["yt"]
            .transpose(0, 1, 3, 2, 4)  # (o, g, j, b, c) -> (o, g, b, j, c)
            .reshape(PADDED_OUT, B_CORE)
            .T.astype(np.float32)
        )
        for r in res.results
    ]
    return np.concatenate(outs, axis=0)[:, :OUTPUT_DIM]


# revision 18
# speedup vs baseline: 1.0050x; 1.0050x over previous
"""Trainium2 Bass kernel for nn_FFMLP (4-layer MLP, hidden=128, relu).

Strategy (pure data parallel, batch sharded 8 ways):
- Feature-major on-chip layout: activations live as [feat, batch]; each layer
  is a K<=128 matmul with the tiny replicated weight stationary and the
  activation stream moving. fp16 operands, fp32 PSUM.
- Layer-batched slab schedule: per slab of 32 chunks, run all of L0, then all
  of L1, ... so consecutive PE matmuls share weights (one LdWeights per layer
  per slab instead of one per matmul -- the interleaved baseline paid ~50us
  of LDWEIGHTS on the PE queue).
- L0 (K=32) packs 4 chunks into concurrent row-tiled matmuls at
  tile_position (32i, 0); L4 (M=16) packs 4 chunks per PSUM bank via column
  tiling (0, 32j). Tiled matmuls execute concurrently on the PE.
- PSUM: 8 banks as two double-buffered 2-bank groups, one owned by the
  Scalar (ACT) engine, one by Vector (DVE). PSUM->SBUF relu evacuation is
  the structural bottleneck (only these two engines can read PSUM);
  a 6:5 ACT:DVE group pattern balances their 0.83 vs 1.10 ns/col rates and
  keeps both ~100% busy without bank starvation.
- Output is packed fp16 in a (group, bank, strip) lexicographic layout so the
  host just reshapes; final cast to fp32 on host.
"""
import sys

if "/opt/trn_rl_repo" not in sys.path:
    sys.path.insert(0, "/opt/trn_rl_repo")

import numpy as np

import concourse.bass as bass
import concourse.mybir as mybir
import concourse.tile as tile

INPUT_DIM = 32
OUTPUT_DIM = 16
HIDDEN = 128
PADDED_OUT = 16
NUM_LAYERS = 4
B = 524288
N_CORES = 8
B_CORE = B // N_CORES  # 65536
CHUNK = 512
N_CHUNKS = B_CORE // CHUNK  # 128
SLAB = 64  # chunks per slab (layer-batched unit)
N_SLABS = N_CHUNKS // SLAB  # 4
QUADS_PER_SLAB = SLAB // 4  # 8 (4-chunk quads for L0 row tiling)
GROUPS_PER_SLAB = SLAB // 2  # 16 2-chunk PSUM groups per layer phase
L4_GROUPS_PER_SLAB = SLAB // 8  # 4 (8 chunks of output per 2-bank group)
N_L4_GROUPS = N_CHUNKS // 8  # 16

fp16 = mybir.dt.float16
fp32 = mybir.dt.float32
RELU = mybir.ActivationFunctionType.Relu

# evac engine pattern: 6 ACT : 5 DVE matches the measured ~1020 : ~1205 ns
# per-op busy times (both engines' spans balance at ~157us)
EVAC_PAT = "ADADADADADA"


def _split_waits(nc, max_waits=1):
    """walrus in this image rejects >1 semaphore wait per instruction on some
    formats; split excess waits onto preceding NOPs on the same engine queue
    (queues are in-order, so semantics are preserved)."""
    n_new = 0
    for bb in nc.main_func.blocks:
        out_list = []
        changed = False
        for ins in bb.instructions:
            si = ins.sync_info
            if si is not None and si.on_wait and len(si.on_wait) > max_waits:
                waits = list(si.on_wait)
                extra, keep = waits[:-max_waits], waits[-max_waits:]
                while extra:
                    chunk, extra = extra[:max_waits], extra[max_waits:]
                    n_new += 1
                    nop = mybir.InstNoOp(name=f"I-waitsplit-{n_new}", ins=[], outs=[])
                    nop.engine = ins.engine
                    nop.sync_info = mybir.SyncInfo(on_wait=chunk, on_update=[])
                    out_list.append(nop)
                ins.sync_info = mybir.SyncInfo(on_wait=keep, on_update=si.on_update)
                changed = True
            out_list.append(ins)
        if changed:
            bb.instructions = out_list
    return n_new


def _ldw_rect(ins):
    """PE-array rectangle (r0, r1, c0, c1) occupied by an InstLdweights."""
    tp = ins.tile_position
    ts = getattr(ins, "tile_size", None)
    r0, c0 = (tp if tp else (0, 0))
    if ts:
        rows, cols = ts
    else:
        rows, cols = 128, 128
    return (r0, r0 + rows, c0, c0 + cols)


def _dedup_ldweights(nc):
    """Tile emits an explicit InstLdweights before every matmul. Weights at a
    given tile rectangle stay resident until an overlapping load clobbers
    them, so replace reloads of already-resident weights with NOPs (keeping
    sync_info). Tracks residency per array rectangle, which handles the
    alternating tile positions of the row/col-tiled L0/L4 phases."""
    n = 0
    for bb in nc.main_func.blocks:
        il = list(bb.instructions)
        live = {}  # rect -> content key
        changed = False
        for idx, ins in enumerate(il):
            if ins.engine != mybir.EngineType.PE:
                continue
            if isinstance(ins, mybir.InstLdweights):
                rect = _ldw_rect(ins)
                key = (
                    repr(ins.ins[0]),
                    str(ins.tile_position),
                    str(getattr(ins, "tile_size", None)),
                    str(ins.perf_mode),
                    bool(ins.is_transpose),
                )
                if live.get(rect) == key:
                    nop = mybir.InstNoOp(name=ins.name, ins=[], outs=[])
                    nop.engine = ins.engine
                    nop.sync_info = ins.sync_info
                    il[idx] = nop
                    changed = True
                    n += 1
                else:
                    r0, r1, c0, c1 = rect
                    for other in list(live):
                        o0, o1, p0, p1 = other
                        if r0 < o1 and o0 < r1 and c0 < p1 and p0 < c1:
                            del live[other]
                    live[rect] = key
        if changed:
            bb.instructions = il
    return n


def build(n_slabs=N_SLABS):
    nc = bass.Bass()
    n_chunks = n_slabs * SLAB
    # xt4: quad-strip layout -- xt4[32*i + f, q*CHUNK + c] = x.T[f, (4q+i)*CHUNK + c]
    # so each quad of 4 chunks feeds 4 concurrent row-tiled K=32 L0 matmuls.
    xt = nc.declare_dram_parameter(
        "xt", [4 * INPUT_DIM, n_chunks * CHUNK // 4], fp16, isOutput=False
    )
    wd = nc.declare_dram_parameter(
        "wd", [HIDDEN, 4 * HIDDEN + PADDED_OUT], fp16, isOutput=False
    )
    # yt[o, g, j, b, c] = y.T[o, (8g + 4b + j)*CHUNK + c] -- the (j, b) order
    # lets one rearranged DMA per L4 group write all 4 partition strips;
    # host transposes (0,1,3,2,4) and reshapes to [16, B_CORE].
    n_l4_groups = n_chunks // 8
    yt = nc.declare_dram_parameter(
        "yt", [PADDED_OUT, n_l4_groups, 4, 2, CHUNK], fp16, isOutput=True
    )

    with tile.TileContext(nc) as tc:
        with (
            tc.tile_pool(name="wp", bufs=1) as wp,
            tc.tile_pool(name="io", bufs=1) as io,
            tc.tile_pool(name="hp", bufs=1) as hp,
            tc.tile_pool(name="ps", bufs=1, space="PSUM") as ps,
        ):
            wall = wp.tile(
                [HIDDEN, 4 * HIDDEN + PADDED_OUT], fp16, tag="wall", name="wall"
            )
            nc.sync.dma_start(out=wall, in_=wd[:, :])
            w0s = wall[:, 0:HIDDEN]
            w1s = wall[:, HIDDEN : 2 * HIDDEN]
            w2s = wall[:, 2 * HIDDEN : 3 * HIDDEN]
            w3s = wall[:, 3 * HIDDEN : 4 * HIDDEN]
            w4s = wall[:, 4 * HIDDEN : 4 * HIDDEN + PADDED_OUT]

            # hidden activation ping-pong buffers, one slab each
            hA = hp.tile([HIDDEN, SLAB * CHUNK], fp16, tag="hA", name="hA")
            hB = hp.tile([HIDDEN, SLAB * CHUNK], fp16, tag="hB", name="hB")

            SLAB_COLS = SLAB * CHUNK // 4  # xt cols per slab (quad-packed)

            xs_tiles = {}

            def fetch_slab(s):
                if s >= n_slabs:
                    return
                xs = io.tile(
                    [4 * INPUT_DIM, SLAB_COLS], fp16, tag="xin", bufs=2, name="xs"
                )
                # quarter-DMAs so L0 of the first quads can start before the
                # whole slab lands (subtile deps gate on each DMA separately)
                q4 = SLAB_COLS // 4
                for k in range(4):
                    nc.sync.dma_start(
                        out=xs[:, k * q4 : (k + 1) * q4],
                        in_=xt[
                            :,
                            s * SLAB_COLS + k * q4 : s * SLAB_COLS + (k + 1) * q4,
                        ],
                    )
                xs_tiles[s] = xs

            fetch_slab(0)
            # short HAM warm-up while the first input quarter lands
            pwarm = ps.tile([HIDDEN, 2 * CHUNK], fp32, tag="pg", bufs=4, name="pwarm")
            for _ in range(8):
                nc.tensor.matmul(
                    pwarm[:, 0:HIDDEN], w1s[:, :], w2s[:, 0:HIDDEN],
                    start=True, stop=True,
                )
            fetch_slab(1)

            evac_state = {"i": 0}

            def evac(dst, src, relu):
                eng = EVAC_PAT[evac_state["i"] % len(EVAC_PAT)]
                evac_state["i"] += 1
                if eng == "A":
                    if relu:
                        nc.scalar.activation(dst, src, RELU)
                    else:
                        nc.scalar.copy(out=dst, in_=src)
                else:
                    if relu:
                        nc.vector.tensor_scalar_max(dst, src, 0.0)
                    else:
                        nc.vector.tensor_copy(dst, src)

            def psum_group():
                # one 4-deep ring of 2-bank groups: the PE can run up to 4
                # groups ahead no matter which evac engine is lagging
                return ps.tile([HIDDEN, 2 * CHUNK], fp32, tag="pg", bufs=4, name="pg")

            def emit_l4(s, gg):
                # L4: M=16, 4-way col tiling packs 4 chunks/bank; a 2-bank
                # group holds 8 chunks of output.
                g = psum_group()
                for b in range(2):
                    for j in range(4):
                        c = 8 * gg + 4 * b + j
                        nc.tensor.matmul(
                            g[32 * j : 32 * j + PADDED_OUT, b * CHUNK : (b + 1) * CHUNK],
                            w4s[:, :],
                            hB[:, c * CHUNK : (c + 1) * CHUNK],
                            start=True, stop=True,
                            tile_position=(0, 32 * j),
                        )
                osb = io.tile([HIDDEN, 2 * CHUNK], fp16, tag="osb", bufs=8, name="osb")
                evac(osb[:, :], g[:, :], False)
                g_abs = s * L4_GROUPS_PER_SLAB + gg
                for j in range(4):
                    eng = nc.sync if j < 2 else nc.gpsimd
                    eng.dma_start(
                        out=yt[:, g_abs : g_abs + 1, j : j + 1, :, :],
                        in_=osb[32 * j : 32 * j + PADDED_OUT, :],
                    )

            for s in range(n_slabs):
                xs = xs_tiles.pop(s)
                # ---- L0: K=32, 4-way row tiling; one quad = 4 concurrent MMs
                # spanning two 2-bank groups. The previous slab's L4 groups
                # (which depend only on the old hB) are interleaved here so
                # the evac engines never starve at the slab boundary.
                for q in range(QUADS_PER_SLAB):
                    g0 = psum_group()
                    for i in range(2):
                        nc.tensor.matmul(
                            g0[:, i * CHUNK : (i + 1) * CHUNK],
                            w0s[32 * i : 32 * i + INPUT_DIM, :],
                            xs[32 * i : 32 * i + INPUT_DIM, q * CHUNK : (q + 1) * CHUNK],
                            start=True, stop=True,
                            tile_position=(32 * i, 0),
                        )
                    evac(hA[:, (4 * q) * CHUNK : (4 * q + 2) * CHUNK], g0[:, :], True)
                    g1 = psum_group()
                    for i in range(2, 4):
                        nc.tensor.matmul(
                            g1[:, (i - 2) * CHUNK : (i - 1) * CHUNK],
                            w0s[32 * i : 32 * i + INPUT_DIM, :],
                            xs[32 * i : 32 * i + INPUT_DIM, q * CHUNK : (q + 1) * CHUNK],
                            start=True, stop=True,
                            tile_position=(32 * i, 0),
                        )
                    evac(hA[:, (4 * q + 2) * CHUNK : (4 * q + 4) * CHUNK], g1[:, :], True)
                # prefetch input two slabs ahead (slabs 0/1 fetched up front)
                if s + 2 < n_slabs:
                    fetch_slab(s + 2)

                # ---- L1..L3: full-array matmuls, 2 chunks per PSUM group.
                # L4 group gg is interleaved into L3 two groups after its
                # source h4 groups (4gg..4gg+3) are emitted, keeping the evac
                # stream dense through the slab boundary and the kernel tail.
                for li, (ws, hin, hout) in enumerate(
                    ((w1s, hA, hB), (w2s, hB, hA), (w3s, hA, hB))
                ):
                    for t in range(GROUPS_PER_SLAB):
                        g = psum_group()
                        for k in range(2):
                            c = 2 * t + k
                            nc.tensor.matmul(
                                g[:, k * CHUNK : (k + 1) * CHUNK],
                                ws[:, :],
                                hin[:, c * CHUNK : (c + 1) * CHUNK],
                                start=True, stop=True,
                            )
                        evac(
                            hout[:, (2 * t) * CHUNK : (2 * t + 2) * CHUNK], g[:, :], True
                        )
                        if li == 2 and t >= 6 and (t - 6) % 4 == 0:
                            emit_l4(s, (t - 6) // 4)
                    if li == 2:
                        for gg in range((GROUPS_PER_SLAB - 6 + 3) // 4, L4_GROUPS_PER_SLAB):
                            emit_l4(s, gg)
    _dedup_ldweights(nc)
    _split_waits(nc)
    return nc


def _split_weights(weights):
    ws = []
    off = 0
    ws.append(weights[off : off + HIDDEN * INPUT_DIM].reshape(HIDDEN, INPUT_DIM))
    off += HIDDEN * INPUT_DIM
    for _ in range(NUM_LAYERS - 1):
        ws.append(weights[off : off + HIDDEN * HIDDEN].reshape(HIDDEN, HIDDEN))
        off += HIDDEN * HIDDEN
    ws.append(weights[off : off + PADDED_OUT * HIDDEN].reshape(PADDED_OUT, HIDDEN))
    return ws


_NC_CACHE = {}


def make_in_maps(inputs: np.ndarray, weights: np.ndarray):
    ws = _split_weights(np.asarray(weights, dtype=np.float32))
    # stationary operands are lhsT = [K_in, M_out] = W.T; W0.T is stacked
    # four times for the four row-tiled strips.
    w0t = np.ascontiguousarray(ws[0].T).astype(np.float16)
    wd = np.concatenate(
        [
            np.concatenate([w0t, w0t, w0t, w0t], axis=0),  # [128, 128]
            np.ascontiguousarray(ws[1].T).astype(np.float16),
            np.ascontiguousarray(ws[2].T).astype(np.float16),
            np.ascontiguousarray(ws[3].T).astype(np.float16),
            np.ascontiguousarray(ws[4].T).astype(np.float16),  # [128, 16]
        ],
        axis=1,
    )
    wmaps = {"wd": np.ascontiguousarray(wd)}
    in_maps = []
    for i in range(N_CORES):
        xc = inputs[i * B_CORE : (i + 1) * B_CORE]
        xtc = np.ascontiguousarray(xc.T).astype(np.float16)  # [32, B_CORE]
        # quad-strip layout: [128, B_CORE//4]
        xt4 = np.ascontiguousarray(
            xtc.reshape(INPUT_DIM, B_CORE // (4 * CHUNK), 4, CHUNK)
            .transpose(2, 0, 1, 3)
            .reshape(4 * INPUT_DIM, B_CORE // 4)
        )
        in_maps.append({"xt": xt4, **wmaps})
    return in_maps


def kernel(inputs: np.ndarray, weights: np.ndarray) -> np.ndarray:
    from concourse.bass_utils import run_bass_kernel_spmd

    assert inputs.shape == (B, INPUT_DIM), inputs.shape
    in_maps = make_in_maps(inputs, weights)
    if "nc" not in _NC_CACHE:
        _NC_CACHE["nc"] = build()
    nc = _NC_CACHE["nc"]
    res = run_bass_kernel_spmd(nc, in_maps, list(range(N_CORES)))
    outs = [
        np.ascontiguousarray(
            r["yt"]
            .transpose(0, 1, 3, 2, 4)  # (o, g, j, b, c) -> (o, g, b, j, c)
            .reshape(PADDED_OUT, B_CORE)
            .T.astype(np.float32)
        )
        for r in res.results
    ]
    return np.concatenate(outs, axis=0)[:, :OUTPUT_DIM]


# revision 20
# speedup vs baseline: 1.0091x; 1.0041x over previous
"""Trainium2 Bass kernel for nn_FFMLP (4-layer MLP, hidden=128, relu).

Strategy (pure data parallel, batch sharded 8 ways):
- Feature-major on-chip layout: activations live as [feat, batch]; each layer
  is a K<=128 matmul with the tiny replicated weight stationary and the
  activation stream moving. fp16 operands, fp32 PSUM.
- Layer-batched slab schedule: per slab of 32 chunks, run all of L0, then all
  of L1, ... so consecutive PE matmuls share weights (one LdWeights per layer
  per slab instead of one per matmul -- the interleaved baseline paid ~50us
  of LDWEIGHTS on the PE queue).
- L0 (K=32) packs 4 chunks into concurrent row-tiled matmuls at
  tile_position (32i, 0); L4 (M=16) packs 4 chunks per PSUM bank via column
  tiling (0, 32j). Tiled matmuls execute concurrently on the PE.
- PSUM: 8 banks as one 4-deep ring of 2-bank groups. PSUM->SBUF relu
  evacuation is the structural bottleneck (only ScalarE/VectorE can read
  PSUM); a 6:5 ACT:DVE group pattern matches their measured ~1.02 vs
  ~1.21 us per-group costs and keeps both engines ~97% busy.
- Each slab's L4 is interleaved into its own L3 phase (two groups of lag)
  so the evacuation stream stays dense across slab boundaries and the
  kernel tail; output strip DMAs are split across the SP and GpSimd
  queues so triggers never serialize behind each other.
- Output is packed fp16 as yt[o, group, strip, bank, c]; host transposes,
  reshapes and casts to fp32.
"""
import sys

if "/opt/trn_rl_repo" not in sys.path:
    sys.path.insert(0, "/opt/trn_rl_repo")

import numpy as np

import concourse.bass as bass
import concourse.mybir as mybir
import concourse.tile as tile

INPUT_DIM = 32
OUTPUT_DIM = 16
HIDDEN = 128
PADDED_OUT = 16
NUM_LAYERS = 4
B = 524288
N_CORES = 8
B_CORE = B // N_CORES  # 65536
CHUNK = 512
N_CHUNKS = B_CORE // CHUNK  # 128
SLAB = 64  # chunks per slab (layer-batched unit)
N_SLABS = N_CHUNKS // SLAB  # 4
QUADS_PER_SLAB = SLAB // 4  # 8 (4-chunk quads for L0 row tiling)
GROUPS_PER_SLAB = SLAB // 2  # 16 2-chunk PSUM groups per layer phase
L4_GROUPS_PER_SLAB = SLAB // 8  # 4 (8 chunks of output per 2-bank group)
N_L4_GROUPS = N_CHUNKS // 8  # 16

fp16 = mybir.dt.float16
fp32 = mybir.dt.float32
RELU = mybir.ActivationFunctionType.Relu

# evac engine pattern: 6 ACT : 5 DVE matches the measured ~1020 : ~1205 ns
# per-op busy times (both engines' spans balance at ~157us)
EVAC_PAT = "ADADADADADA"


def _split_waits(nc, max_waits=1):
    """walrus in this image rejects >1 semaphore wait per instruction on some
    formats; split excess waits onto preceding NOPs on the same engine queue
    (queues are in-order, so semantics are preserved)."""
    n_new = 0
    for bb in nc.main_func.blocks:
        out_list = []
        changed = False
        for ins in bb.instructions:
            si = ins.sync_info
            if si is not None and si.on_wait and len(si.on_wait) > max_waits:
                waits = list(si.on_wait)
                extra, keep = waits[:-max_waits], waits[-max_waits:]
                while extra:
                    chunk, extra = extra[:max_waits], extra[max_waits:]
                    n_new += 1
                    nop = mybir.InstNoOp(name=f"I-waitsplit-{n_new}", ins=[], outs=[])
                    nop.engine = ins.engine
                    nop.sync_info = mybir.SyncInfo(on_wait=chunk, on_update=[])
                    out_list.append(nop)
                ins.sync_info = mybir.SyncInfo(on_wait=keep, on_update=si.on_update)
                changed = True
            out_list.append(ins)
        if changed:
            bb.instructions = out_list
    return n_new


def _ldw_rect(ins):
    """PE-array rectangle (r0, r1, c0, c1) occupied by an InstLdweights."""
    tp = ins.tile_position
    ts = getattr(ins, "tile_size", None)
    r0, c0 = (tp if tp else (0, 0))
    if ts:
        rows, cols = ts
    else:
        rows, cols = 128, 128
    return (r0, r0 + rows, c0, c0 + cols)


def _dedup_ldweights(nc):
    """Tile emits an explicit InstLdweights before every matmul. Weights at a
    given tile rectangle stay resident until an overlapping load clobbers
    them, so replace reloads of already-resident weights with NOPs (keeping
    sync_info). Tracks residency per array rectangle, which handles the
    alternating tile positions of the row/col-tiled L0/L4 phases."""
    n = 0
    for bb in nc.main_func.blocks:
        il = list(bb.instructions)
        live = {}  # rect -> content key
        changed = False
        for idx, ins in enumerate(il):
            if ins.engine != mybir.EngineType.PE:
                continue
            if isinstance(ins, mybir.InstLdweights):
                rect = _ldw_rect(ins)
                key = (
                    repr(ins.ins[0]),
                    str(ins.tile_position),
                    str(getattr(ins, "tile_size", None)),
                    str(ins.perf_mode),
                    bool(ins.is_transpose),
                )
                if live.get(rect) == key:
                    nop = mybir.InstNoOp(name=ins.name, ins=[], outs=[])
                    nop.engine = ins.engine
                    nop.sync_info = ins.sync_info
                    il[idx] = nop
                    changed = True
                    n += 1
                else:
                    r0, r1, c0, c1 = rect
                    for other in list(live):
                        o0, o1, p0, p1 = other
                        if r0 < o1 and o0 < r1 and c0 < p1 and p0 < c1:
                            del live[other]
                    live[rect] = key
        if changed:
            bb.instructions = il
    return n


def build(n_slabs=N_SLABS):
    nc = bass.Bass()
    n_chunks = n_slabs * SLAB
    # xt4: quad-strip layout -- xt4[32*i + f, q*CHUNK + c] = x.T[f, (4q+i)*CHUNK + c]
    # so each quad of 4 chunks feeds 4 concurrent row-tiled K=32 L0 matmuls.
    xt = nc.declare_dram_parameter(
        "xt", [4 * INPUT_DIM, n_chunks * CHUNK // 4], fp16, isOutput=False
    )
    wd = nc.declare_dram_parameter(
        "wd", [HIDDEN, 4 * HIDDEN + PADDED_OUT], fp16, isOutput=False
    )
    # yt[o, g, j, b, c] = y.T[o, (8g + 4b + j)*CHUNK + c] -- the (j, b) order
    # lets one rearranged DMA per L4 group write all 4 partition strips;
    # host transposes (0,1,3,2,4) and reshapes to [16, B_CORE].
    n_l4_groups = n_chunks // 8
    yt = nc.declare_dram_parameter(
        "yt", [PADDED_OUT, n_l4_groups, 4, 2, CHUNK], fp16, isOutput=True
    )

    with tile.TileContext(nc) as tc:
        with (
            tc.tile_pool(name="wp", bufs=1) as wp,
            tc.tile_pool(name="io", bufs=1) as io,
            tc.tile_pool(name="hp", bufs=1) as hp,
            tc.tile_pool(name="ps", bufs=1, space="PSUM") as ps,
        ):
            wall = wp.tile(
                [HIDDEN, 4 * HIDDEN + PADDED_OUT], fp16, tag="wall", name="wall"
            )
            nc.sync.dma_start(out=wall, in_=wd[:, :])
            w0s = wall[:, 0:HIDDEN]
            w1s = wall[:, HIDDEN : 2 * HIDDEN]
            w2s = wall[:, 2 * HIDDEN : 3 * HIDDEN]
            w3s = wall[:, 3 * HIDDEN : 4 * HIDDEN]
            w4s = wall[:, 4 * HIDDEN : 4 * HIDDEN + PADDED_OUT]

            # hidden activation ping-pong buffers, one slab each
            hA = hp.tile([HIDDEN, SLAB * CHUNK], fp16, tag="hA", name="hA")
            hB = hp.tile([HIDDEN, SLAB * CHUNK], fp16, tag="hB", name="hB")

            SLAB_COLS = SLAB * CHUNK // 4  # xt cols per slab (quad-packed)

            xs_tiles = {}

            def fetch_slab(s):
                if s >= n_slabs:
                    return
                xs = io.tile(
                    [4 * INPUT_DIM, SLAB_COLS], fp16, tag="xin", bufs=2, name="xs"
                )
                # quarter-DMAs so L0 of the first quads can start before the
                # whole slab lands (subtile deps gate on each DMA separately)
                q4 = SLAB_COLS // 4
                for k in range(4):
                    nc.sync.dma_start(
                        out=xs[:, k * q4 : (k + 1) * q4],
                        in_=xt[
                            :,
                            s * SLAB_COLS + k * q4 : s * SLAB_COLS + (k + 1) * q4,
                        ],
                    )
                xs_tiles[s] = xs

            fetch_slab(0)
            # HAM warm-up on a memset dummy tile: no DMA dependency, so the
            # PE starts right after the engine preamble and accumulates the
            # ~3us of continuous busy needed to reach 2.4 GHz before the
            # first real fill (8 weight-gated MMs left it at 1.2 GHz through
            # slab 0, making fills slower than the ACT/DVE drain).
            dummy = io.tile([HIDDEN, CHUNK], fp16, tag="dum", name="dummy")
            nc.gpsimd.memset(dummy[:, :], 0.0)
            pwarm = ps.tile([HIDDEN, 2 * CHUNK], fp32, tag="pg", bufs=4, name="pwarm")
            for _ in range(16):
                nc.tensor.matmul(
                    pwarm[:, 0:CHUNK], dummy[:, 0:HIDDEN], dummy[:, :],
                    start=True, stop=True,
                )
            fetch_slab(1)

            evac_state = {"i": 0}

            def evac(dst, src, relu):
                eng = EVAC_PAT[evac_state["i"] % len(EVAC_PAT)]
                evac_state["i"] += 1
                if eng == "A":
                    if relu:
                        nc.scalar.activation(dst, src, RELU)
                    else:
                        nc.scalar.copy(out=dst, in_=src)
                else:
                    if relu:
                        nc.vector.tensor_scalar_max(dst, src, 0.0)
                    else:
                        nc.vector.tensor_copy(dst, src)

            def psum_group():
                # one 4-deep ring of 2-bank groups: the PE can run up to 4
                # groups ahead no matter which evac engine is lagging
                return ps.tile([HIDDEN, 2 * CHUNK], fp32, tag="pg", bufs=4, name="pg")

            def emit_l4(s, gg):
                # L4: M=16, 4-way col tiling packs 4 chunks/bank; a 2-bank
                # group holds 8 chunks of output.
                g = psum_group()
                for b in range(2):
                    for j in range(4):
                        c = 8 * gg + 4 * b + j
                        nc.tensor.matmul(
                            g[32 * j : 32 * j + PADDED_OUT, b * CHUNK : (b + 1) * CHUNK],
                            w4s[:, :],
                            hB[:, c * CHUNK : (c + 1) * CHUNK],
                            start=True, stop=True,
                            tile_position=(0, 32 * j),
                        )
                osb = io.tile([HIDDEN, 2 * CHUNK], fp16, tag="osb", bufs=8, name="osb")
                evac(osb[:, :], g[:, :], False)
                g_abs = s * L4_GROUPS_PER_SLAB + gg
                for j in range(4):
                    eng = nc.sync if j < 2 else nc.gpsimd
                    eng.dma_start(
                        out=yt[:, g_abs : g_abs + 1, j : j + 1, :, :],
                        in_=osb[32 * j : 32 * j + PADDED_OUT, :],
                    )

            for s in range(n_slabs):
                xs = xs_tiles.pop(s)
                # ---- L0: K=32, 4-way row tiling; one quad = 4 concurrent MMs
                # spanning two 2-bank groups. The previous slab's L4 groups
                # (which depend only on the old hB) are interleaved here so
                # the evac engines never starve at the slab boundary.
                for q in range(QUADS_PER_SLAB):
                    g0 = psum_group()
                    for i in range(2):
                        nc.tensor.matmul(
                            g0[:, i * CHUNK : (i + 1) * CHUNK],
                            w0s[32 * i : 32 * i + INPUT_DIM, :],
                            xs[32 * i : 32 * i + INPUT_DIM, q * CHUNK : (q + 1) * CHUNK],
                            start=True, stop=True,
                            tile_position=(32 * i, 0),
                        )
                    evac(hA[:, (4 * q) * CHUNK : (4 * q + 2) * CHUNK], g0[:, :], True)
                    g1 = psum_group()
                    for i in range(2, 4):
                        nc.tensor.matmul(
                            g1[:, (i - 2) * CHUNK : (i - 1) * CHUNK],
                            w0s[32 * i : 32 * i + INPUT_DIM, :],
                            xs[32 * i : 32 * i + INPUT_DIM, q * CHUNK : (q + 1) * CHUNK],
                            start=True, stop=True,
                            tile_position=(32 * i, 0),
                        )
                    evac(hA[:, (4 * q + 2) * CHUNK : (4 * q + 4) * CHUNK], g1[:, :], True)
                # prefetch input two slabs ahead (slabs 0/1 fetched up front)
                if s + 2 < n_slabs:
                    fetch_slab(s + 2)

                # ---- L1..L3: full-array matmuls, 2 chunks per PSUM group.
                # L4 group gg is interleaved into L3 two groups after its
                # source h4 groups (4gg..4gg+3) are emitted, keeping the evac
                # stream dense through the slab boundary and the kernel tail.
                for li, (ws, hin, hout) in enumerate(
                    ((w1s, hA, hB), (w2s, hB, hA), (w3s, hA, hB))
                ):
                    for t in range(GROUPS_PER_SLAB):
                        g = psum_group()
                        for k in range(2):
                            c = 2 * t + k
                            nc.tensor.matmul(
                                g[:, k * CHUNK : (k + 1) * CHUNK],
                                ws[:, :],
                                hin[:, c * CHUNK : (c + 1) * CHUNK],
                                start=True, stop=True,
                            )
                        evac(
                            hout[:, (2 * t) * CHUNK : (2 * t + 2) * CHUNK], g[:, :], True
                        )
                        if li == 2 and t >= 8 and (t - 8) % 4 == 0:
                            emit_l4(s, (t - 8) // 4)
                    if li == 2:
                        for gg in range((GROUPS_PER_SLAB - 8 + 3) // 4, L4_GROUPS_PER_SLAB):
                            emit_l4(s, gg)
    _dedup_ldweights(nc)
    _split_waits(nc)
    return nc


def _split_weights(weights):
    ws = []
    off = 0
    ws.append(weights[off : off + HIDDEN * INPUT_DIM].reshape(HIDDEN, INPUT_DIM))
    off += HIDDEN * INPUT_DIM
    for _ in range(NUM_LAYERS - 1):
        ws.append(weights[off : off + HIDDEN * HIDDEN].reshape(HIDDEN, HIDDEN))
        off += HIDDEN * HIDDEN
    ws.append(weights[off : off + PADDED_OUT * HIDDEN].reshape(PADDED_OUT, HIDDEN))
    return ws


_NC_CACHE = {}


def make_in_maps(inputs: np.ndarray, weights: np.ndarray):
    ws = _split_weights(np.asarray(weights, dtype=np.float32))
    # stationary operands are lhsT = [K_in, M_out] = W.T; W0.T is stacked
    # four times for the four row-tiled strips.
    w0t = np.ascontiguousarray(ws[0].T).astype(np.float16)
    wd = np.concatenate(
        [
            np.concatenate([w0t, w0t, w0t, w0t], axis=0),  # [128, 128]
            np.ascontiguousarray(ws[1].T).astype(np.float16),
            np.ascontiguousarray(ws[2].T).astype(np.float16),
            np.ascontiguousarray(ws[3].T).astype(np.float16),
            np.ascontiguousarray(ws[4].T).astype(np.float16),  # [128, 16]
        ],
        axis=1,
    )
    wmaps = {"wd": np.ascontiguousarray(wd)}
    in_maps = []
    for i in range(N_CORES):
        xc = inputs[i * B_CORE : (i + 1) * B_CORE]
        xtc = np.ascontiguousarray(xc.T).astype(np.float16)  # [32, B_CORE]
        # quad-strip layout: [128, B_CORE//4]
        xt4 = np.ascontiguousarray(
            xtc.reshape(INPUT_DIM, B_CORE // (4 * CHUNK), 4, CHUNK)
            .transpose(2, 0, 1, 3)
            .reshape(4 * INPUT_DIM, B_CORE // 4)
        )
        in_maps.append({"xt": xt4, **wmaps})
    return in_maps


def kernel(inputs: np.ndarray, weights: np.ndarray) -> np.ndarray:
    from concourse.bass_utils import run_bass_kernel_spmd

    assert inputs.shape == (B, INPUT_DIM), inputs.shape
    in_maps = make_in_maps(inputs, weights)
    if "nc" not in _NC_CACHE:
        _NC_CACHE["nc"] = build()
    nc = _NC_CACHE["nc"]
    res = run_bass_kernel_spmd(nc, in_maps, list(range(N_CORES)))
    outs = [
        np.ascontiguousarray(
            r["yt"]
            .transpose(0, 1, 3, 2, 4)  # (o, g, j, b, c) -> (o, g, b, j, c)
            .reshape(PADDED_OUT, B_CORE)
            .T.astype(np.float32)
        )
        for r in res.results
    ]
    return np.concatenate(outs, axis=0)[:, :OUTPUT_DIM]


# revision 22
# speedup vs baseline: 1.0236x; 1.0144x over previous
"""Trainium2 Bass kernel for nn_FFMLP (4-layer MLP, hidden=128, relu).

Strategy (pure data parallel, batch sharded 8 ways):
- Feature-major on-chip layout: activations live as [feat, batch]; each layer
  is a K<=128 matmul with the tiny replicated weight stationary and the
  activation stream moving. fp16 operands, fp32 PSUM.
- Layer-batched slab schedule: per slab of 32 chunks, run all of L0, then all
  of L1, ... so consecutive PE matmuls share weights (one LdWeights per layer
  per slab instead of one per matmul -- the interleaved baseline paid ~50us
  of LDWEIGHTS on the PE queue).
- L0 (K=32) packs 4 chunks into concurrent row-tiled matmuls at
  tile_position (32i, 0); L4 (M=16) packs 4 chunks per PSUM bank via column
  tiling (0, 32j). Tiled matmuls execute concurrently on the PE.
- PSUM: 8 banks as one 4-deep ring of 2-bank groups. PSUM->SBUF relu
  evacuation is the structural bottleneck (only ScalarE/VectorE can read
  PSUM); a 6:5 ACT:DVE group pattern matches their measured ~1.02 vs
  ~1.21 us per-group costs and keeps both engines ~97% busy.
- Each slab's L4 is interleaved into its own L3 phase (two groups of lag)
  so the evacuation stream stays dense across slab boundaries and the
  kernel tail; output strip DMAs are split across the SP and GpSimd
  queues so triggers never serialize behind each other.
- Output is packed fp16 as yt[o, group, strip, bank, c]; host transposes,
  reshapes and casts to fp32.
"""
import sys

if "/opt/trn_rl_repo" not in sys.path:
    sys.path.insert(0, "/opt/trn_rl_repo")

import numpy as np

import concourse.bass as bass
import concourse.mybir as mybir
import concourse.tile as tile

INPUT_DIM = 32
OUTPUT_DIM = 16
HIDDEN = 128
PADDED_OUT = 16
NUM_LAYERS = 4
B = 524288
N_CORES = 8
B_CORE = B // N_CORES  # 65536
CHUNK = 512
N_CHUNKS = B_CORE // CHUNK  # 128
SLAB = 64  # chunks per slab (layer-batched unit)
N_SLABS = N_CHUNKS // SLAB  # 4
QUADS_PER_SLAB = SLAB // 4  # 8 (4-chunk quads for L0 row tiling)
GROUPS_PER_SLAB = SLAB // 2  # 16 2-chunk PSUM groups per layer phase
L4_GROUPS_PER_SLAB = SLAB // 8  # 4 (8 chunks of output per 2-bank group)
N_L4_GROUPS = N_CHUNKS // 8  # 16

fp16 = mybir.dt.float16
fp32 = mybir.dt.float32
RELU = mybir.ActivationFunctionType.Relu

# evac engine pattern: 6 ACT : 5 DVE matches the measured ~1020 : ~1205 ns
# per-op busy times (both engines' spans balance at ~157us)
EVAC_PAT = "ADADADADADA"


def _split_waits(nc, max_waits=1):
    """walrus in this image rejects >1 semaphore wait per instruction on some
    formats; split excess waits onto preceding NOPs on the same engine queue
    (queues are in-order, so semantics are preserved)."""
    n_new = 0
    for bb in nc.main_func.blocks:
        out_list = []
        changed = False
        for ins in bb.instructions:
            si = ins.sync_info
            if si is not None and si.on_wait and len(si.on_wait) > max_waits:
                waits = list(si.on_wait)
                extra, keep = waits[:-max_waits], waits[-max_waits:]
                while extra:
                    chunk, extra = extra[:max_waits], extra[max_waits:]
                    n_new += 1
                    nop = mybir.InstNoOp(name=f"I-waitsplit-{n_new}", ins=[], outs=[])
                    nop.engine = ins.engine
                    nop.sync_info = mybir.SyncInfo(on_wait=chunk, on_update=[])
                    out_list.append(nop)
                ins.sync_info = mybir.SyncInfo(on_wait=keep, on_update=si.on_update)
                changed = True
            out_list.append(ins)
        if changed:
            bb.instructions = out_list
    return n_new


def _ldw_rect(ins):
    """PE-array rectangle (r0, r1, c0, c1) occupied by an InstLdweights."""
    tp = ins.tile_position
    ts = getattr(ins, "tile_size", None)
    r0, c0 = (tp if tp else (0, 0))
    if ts:
        rows, cols = ts
    else:
        rows, cols = 128, 128
    return (r0, r0 + rows, c0, c0 + cols)


def _dedup_ldweights(nc):
    """Tile emits an explicit InstLdweights before every matmul. Weights at a
    given tile rectangle stay resident until an overlapping load clobbers
    them, so replace reloads of already-resident weights with NOPs (keeping
    sync_info). Tracks residency per array rectangle, which handles the
    alternating tile positions of the row/col-tiled L0/L4 phases."""
    n = 0
    for bb in nc.main_func.blocks:
        il = list(bb.instructions)
        live = {}  # rect -> content key
        changed = False
        for idx, ins in enumerate(il):
            if ins.engine != mybir.EngineType.PE:
                continue
            if isinstance(ins, mybir.InstLdweights):
                rect = _ldw_rect(ins)
                key = (
                    repr(ins.ins[0]),
                    str(ins.tile_position),
                    str(getattr(ins, "tile_size", None)),
                    str(ins.perf_mode),
                    bool(ins.is_transpose),
                )
                if live.get(rect) == key:
                    nop = mybir.InstNoOp(name=ins.name, ins=[], outs=[])
                    nop.engine = ins.engine
                    nop.sync_info = ins.sync_info
                    il[idx] = nop
                    changed = True
                    n += 1
                else:
                    r0, r1, c0, c1 = rect
                    for other in list(live):
                        o0, o1, p0, p1 = other
                        if r0 < o1 and o0 < r1 and c0 < p1 and p0 < c1:
                            del live[other]
                    live[rect] = key
        if changed:
            bb.instructions = il
    return n


def build(n_slabs=N_SLABS):
    nc = bass.Bass()
    n_chunks = n_slabs * SLAB
    # xt4: quad-strip layout -- xt4[32*i + f, q*CHUNK + c] = x.T[f, (4q+i)*CHUNK + c]
    # so each quad of 4 chunks feeds 4 concurrent row-tiled K=32 L0 matmuls.
    xt = nc.declare_dram_parameter(
        "xt", [4 * INPUT_DIM, n_chunks * CHUNK // 4], fp16, isOutput=False
    )
    wd = nc.declare_dram_parameter(
        "wd", [HIDDEN, 4 * HIDDEN + PADDED_OUT], fp16, isOutput=False
    )
    # yt[o, g, j, b, c] = y.T[o, (8g + 4b + j)*CHUNK + c] -- the (j, b) order
    # lets one rearranged DMA per L4 group write all 4 partition strips;
    # host transposes (0,1,3,2,4) and reshapes to [16, B_CORE].
    n_l4_groups = n_chunks // 8
    yt = nc.declare_dram_parameter(
        "yt", [PADDED_OUT, n_l4_groups, 4, 2, CHUNK], fp16, isOutput=True
    )

    with tile.TileContext(nc) as tc:
        with (
            tc.tile_pool(name="wp", bufs=1) as wp,
            tc.tile_pool(name="io", bufs=1) as io,
            tc.tile_pool(name="hp", bufs=1) as hp,
            tc.tile_pool(name="ps", bufs=1, space="PSUM") as ps,
        ):
            wall = wp.tile(
                [HIDDEN, 4 * HIDDEN + PADDED_OUT], fp16, tag="wall", name="wall"
            )
            nc.sync.dma_start(out=wall, in_=wd[:, :])
            w0s = wall[:, 0:HIDDEN]
            w1s = wall[:, HIDDEN : 2 * HIDDEN]
            w2s = wall[:, 2 * HIDDEN : 3 * HIDDEN]
            w3s = wall[:, 3 * HIDDEN : 4 * HIDDEN]
            w4s = wall[:, 4 * HIDDEN : 4 * HIDDEN + PADDED_OUT]

            # hidden activation ping-pong buffers, one slab each
            hA = hp.tile([HIDDEN, SLAB * CHUNK], fp16, tag="hA", name="hA")
            hB = hp.tile([HIDDEN, SLAB * CHUNK], fp16, tag="hB", name="hB")

            SLAB_COLS = SLAB * CHUNK // 4  # xt cols per slab (quad-packed)

            xs_tiles = {}

            def fetch_slab(s):
                if s >= n_slabs:
                    return
                xs = io.tile(
                    [4 * INPUT_DIM, SLAB_COLS], fp16, tag="xin", bufs=2, name="xs"
                )
                # quarter-DMAs so L0 of the first quads can start before the
                # whole slab lands (subtile deps gate on each DMA separately);
                # slab 0's first quarter is further split so the very first
                # quad's input lands as early as possible
                q4 = SLAB_COLS // 4
                bounds = ([0, q4 // 2, q4] if s == 0 else [0, q4]) + [
                    2 * q4, 3 * q4, 4 * q4
                ]
                for lo, hi in zip(bounds, bounds[1:]):
                    nc.sync.dma_start(
                        out=xs[:, lo:hi],
                        in_=xt[:, s * SLAB_COLS + lo : s * SLAB_COLS + hi],
                    )
                xs_tiles[s] = xs

            fetch_slab(0)
            # HAM warm-up on a memset dummy tile: no DMA dependency, so the
            # PE starts right after the engine preamble and accumulates the
            # ~3us of continuous busy needed to reach 2.4 GHz before the
            # first real fill (8 weight-gated MMs left it at 1.2 GHz through
            # slab 0, making fills slower than the ACT/DVE drain).
            dummy = io.tile([HIDDEN, CHUNK], fp16, tag="dum", name="dummy")
            nc.vector.memset(dummy[:, :], 0.0)
            pwarm = ps.tile([HIDDEN, 2 * CHUNK], fp32, tag="pg", bufs=4, name="pwarm")
            for _ in range(8):
                nc.tensor.matmul(
                    pwarm[:, 0:CHUNK], dummy[:, 0:HIDDEN], dummy[:, :],
                    start=True, stop=True,
                )
            fetch_slab(1)

            evac_state = {"i": 0}

            def evac(dst, src, relu):
                eng = EVAC_PAT[evac_state["i"] % len(EVAC_PAT)]
                evac_state["i"] += 1
                if eng == "A":
                    if relu:
                        nc.scalar.activation(dst, src, RELU)
                    else:
                        nc.scalar.copy(out=dst, in_=src)
                else:
                    if relu:
                        nc.vector.tensor_scalar_max(dst, src, 0.0)
                    else:
                        nc.vector.tensor_copy(dst, src)

            def psum_group():
                # one 4-deep ring of 2-bank groups: the PE can run up to 4
                # groups ahead no matter which evac engine is lagging
                return ps.tile([HIDDEN, 2 * CHUNK], fp32, tag="pg", bufs=4, name="pg")

            def emit_l4(s, gg):
                # L4: M=16, 4-way col tiling packs 4 chunks/bank; a 2-bank
                # group holds 8 chunks of output.
                g = psum_group()
                for b in range(2):
                    for j in range(4):
                        c = 8 * gg + 4 * b + j
                        nc.tensor.matmul(
                            g[32 * j : 32 * j + PADDED_OUT, b * CHUNK : (b + 1) * CHUNK],
                            w4s[:, :],
                            hB[:, c * CHUNK : (c + 1) * CHUNK],
                            start=True, stop=True,
                            tile_position=(0, 32 * j),
                        )
                osb = io.tile([HIDDEN, 2 * CHUNK], fp16, tag="osb", bufs=8, name="osb")
                evac(osb[:, :], g[:, :], False)
                g_abs = s * L4_GROUPS_PER_SLAB + gg
                for j in range(4):
                    eng = nc.sync if j < 2 else nc.gpsimd
                    eng.dma_start(
                        out=yt[:, g_abs : g_abs + 1, j : j + 1, :, :],
                        in_=osb[32 * j : 32 * j + PADDED_OUT, :],
                    )

            for s in range(n_slabs):
                xs = xs_tiles.pop(s)
                # ---- L0: K=32, 4-way row tiling; one quad = 4 concurrent MMs
                # spanning two 2-bank groups. The previous slab's L4 groups
                # (which depend only on the old hB) are interleaved here so
                # the evac engines never starve at the slab boundary.
                for q in range(QUADS_PER_SLAB):
                    g0 = psum_group()
                    for i in range(2):
                        nc.tensor.matmul(
                            g0[:, i * CHUNK : (i + 1) * CHUNK],
                            w0s[32 * i : 32 * i + INPUT_DIM, :],
                            xs[32 * i : 32 * i + INPUT_DIM, q * CHUNK : (q + 1) * CHUNK],
                            start=True, stop=True,
                            tile_position=(32 * i, 0),
                        )
                    evac(hA[:, (4 * q) * CHUNK : (4 * q + 2) * CHUNK], g0[:, :], True)
                    g1 = psum_group()
                    for i in range(2, 4):
                        nc.tensor.matmul(
                            g1[:, (i - 2) * CHUNK : (i - 1) * CHUNK],
                            w0s[32 * i : 32 * i + INPUT_DIM, :],
                            xs[32 * i : 32 * i + INPUT_DIM, q * CHUNK : (q + 1) * CHUNK],
                            start=True, stop=True,
                            tile_position=(32 * i, 0),
                        )
                    evac(hA[:, (4 * q + 2) * CHUNK : (4 * q + 4) * CHUNK], g1[:, :], True)
                # prefetch input two slabs ahead (slabs 0/1 fetched up front)
                if s + 2 < n_slabs:
                    fetch_slab(s + 2)

                # ---- L1..L3: full-array matmuls, 2 chunks per PSUM group.
                # L4 group gg is interleaved into L3 two groups after its
                # source h4 groups (4gg..4gg+3) are emitted, keeping the evac
                # stream dense through the slab boundary and the kernel tail.
                for li, (ws, hin, hout) in enumerate(
                    ((w1s, hA, hB), (w2s, hB, hA), (w3s, hA, hB))
                ):
                    for t in range(GROUPS_PER_SLAB):
                        g = psum_group()
                        for k in range(2):
                            c = 2 * t + k
                            nc.tensor.matmul(
                                g[:, k * CHUNK : (k + 1) * CHUNK],
                                ws[:, :],
                                hin[:, c * CHUNK : (c + 1) * CHUNK],
                                start=True, stop=True,
                            )
                        evac(
                            hout[:, (2 * t) * CHUNK : (2 * t + 2) * CHUNK], g[:, :], True
                        )
                        if li == 2 and t >= 8 and (t - 8) % 4 == 0:
                            emit_l4(s, (t - 8) // 4)
                    if li == 2:
                        for gg in range((GROUPS_PER_SLAB - 8 + 3) // 4, L4_GROUPS_PER_SLAB):
                            emit_l4(s, gg)
    _dedup_ldweights(nc)
    _split_waits(nc)
    return nc


def _split_weights(weights):
    ws = []
    off = 0
    ws.append(weights[off : off + HIDDEN * INPUT_DIM].reshape(HIDDEN, INPUT_DIM))
    off += HIDDEN * INPUT_DIM
    for _ in range(NUM_LAYERS - 1):
        ws.append(weights[off : off + HIDDEN * HIDDEN].reshape(HIDDEN, HIDDEN))
        off += HIDDEN * HIDDEN
    ws.append(weights[off : off + PADDED_OUT * HIDDEN].reshape(PADDED_OUT, HIDDEN))
    return ws


_NC_CACHE = {}


def make_in_maps(inputs: np.ndarray, weights: np.ndarray):
    ws = _split_weights(np.asarray(weights, dtype=np.float32))
    # stationary operands are lhsT = [K_in, M_out] = W.T; W0.T is stacked
    # four times for the four row-tiled strips.
    w0t = np.ascontiguousarray(ws[0].T).astype(np.float16)
    wd = np.concatenate(
        [
            np.concatenate([w0t, w0t, w0t, w0t], axis=0),  # [128, 128]
            np.ascontiguousarray(ws[1].T).astype(np.float16),
            np.ascontiguousarray(ws[2].T).astype(np.float16),
            np.ascontiguousarray(ws[3].T).astype(np.float16),
            np.ascontiguousarray(ws[4].T).astype(np.float16),  # [128, 16]
        ],
        axis=1,
    )
    wmaps = {"wd": np.ascontiguousarray(wd)}
    in_maps = []
    for i in range(N_CORES):
        xc = inputs[i * B_CORE : (i + 1) * B_CORE]
        xtc = np.ascontiguousarray(xc.T).astype(np.float16)  # [32, B_CORE]
        # quad-strip layout: [128, B_CORE//4]
        xt4 = np.ascontiguousarray(
            xtc.reshape(INPUT_DIM, B_CORE // (4 * CHUNK), 4, CHUNK)
            .transpose(2, 0, 1, 3)
            .reshape(4 * INPUT_DIM, B_CORE // 4)
        )
        in_maps.append({"xt": xt4, **wmaps})
    return in_maps


def kernel(inputs: np.ndarray, weights: np.ndarray) -> np.ndarray:
    from concourse.bass_utils import run_bass_kernel_spmd

    assert inputs.shape == (B, INPUT_DIM), inputs.shape
    in_maps = make_in_maps(inputs, weights)
    if "nc" not in _NC_CACHE:
        _NC_CACHE["nc"] = build()
    nc = _NC_CACHE["nc"]
    res = run_bass_kernel_spmd(nc, in_maps, list(range(N_CORES)))
    outs = [
        np.ascontiguousarray(
            r["yt"]
            .transpose(0, 1, 3, 2, 4)  # (o, g, j, b, c) -> (o, g, b, j, c)
            .reshape(PADDED_OUT, B_CORE)
            .T.astype(np.float32)
        )
        for r in res.results
    ]
    return np.concatenate(outs, axis=0)[:, :OUTPUT_DIM]
